# revision 7
# baseline (speedup 1.0000x reference)
"""ChromosomeEmbedding kernel for 8x Trainium2 NeuronCores.

Computes out[b, j, d] = ce[chr[b]-1, d] for b in [0,512), j in [0,2001),
d in [0,128). Data-parallel: batch sharded 64 samples/core across 8 cores;
the 24x128 table ce is replicated.

Per-core device program:
  1. dma chr shard (int32) into SBUF in dma_gather's wrapped idx layout
  2. subtract 1, cast to int16 -> gather indices
  3. dma_gather: partitions 0..127 each receive ce[chr[p % 64]-1] (the 64
     sample rows, twice)
  4. 6 doubling copies replicate each row 64x along the free dim
  5. 32 large DMAs stream the [64, 2001, 128] output shard to HBM
     (partition half 0:64 writes bins [t*128, t*128+64), half 64:128
     writes bins [t*128+64, t*128+128))
"""

import functools

import numpy as np

from concourse import bacc, bass, mybir, tile
from concourse.bass_utils import run_bass_kernel_spmd

N_CORES = 8
BS = 512
BPC = BS // N_CORES  # 64 samples per core
NBIN = 2001
DIM = 128
N_CHR = 24
REP = 64  # replicated copies of each row held in SBUF
F32 = mybir.dt.float32


@functools.lru_cache(maxsize=1)
def build_nc():
    nc = bacc.Bacc("TRN2", target_bir_lowering=False)

    chr_h = nc.declare_dram_parameter("chr", [128, 8], mybir.dt.int16, isOutput=False)
    ce_h = nc.declare_dram_parameter("ce", [N_CHR, DIM], F32, isOutput=False)
    out_h = nc.declare_dram_parameter("out", [BPC, NBIN, DIM], F32, isOutput=True)

    with tile.TileContext(nc) as tc:
        with tc.tile_pool(name="pool", bufs=1) as pool:
            c16 = pool.tile([128, 8], mybir.dt.int16, tag="c16")
            idx16 = pool.tile([128, 8], mybir.dt.int16, tag="idx")
            rep = pool.tile([128, REP, DIM], F32, tag="rep")

            # Gather indices: dma_gather reads idx i from partition i%16,
            # column i//16, replicated across the 8 GpSimd-core partition
            # groups. chr arrives already in that wrapped [128, 8] layout;
            # the -1 is applied on device in-width (int16).
            nc.sync.dma_start(out=c16[:, :], in_=chr_h[:, :])
            nc.vector.tensor_scalar(
                out=idx16[:, :],
                in0=c16[:, :],
                scalar1=1,
                scalar2=None,
                op0=mybir.AluOpType.subtract,
            )

            # rows: partition p <- ce[chr[p % 64] - 1]
            nc.gpsimd.dma_gather(
                out_ap=rep[:, 0:1, :],
                in_ap=ce_h[:, :],
                idxs_ap=idx16[:, :],
                num_idxs=128,
                num_idxs_reg=128,
                elem_size=DIM,
            )

            # Replicate each partition's row REP times along the free dim.
            w = 1
            while w < REP:
                nc.vector.tensor_copy(out=rep[:, w : 2 * w, :], in_=rep[:, 0:w, :])
                w *= 2

            # Stream the output: pass t covers bins [t*128, (t+1)*128).
            dma_engines = [nc.sync, nc.scalar]
            nfull = NBIN // (2 * REP)  # 15 full passes of 128 bins
            for t in range(nfull):
                j0 = t * 2 * REP
                dma_engines[t % 2].dma_start(
                    out=out_h[:, j0 : j0 + REP, :], in_=rep[0:BPC, :, :]
                )
                dma_engines[(t + 1) % 2].dma_start(
                    out=out_h[:, j0 + REP : j0 + 2 * REP, :], in_=rep[BPC:128, :, :]
                )
            # Remainder: 81 bins = 41 from the low half + 40 from the high.
            j0 = nfull * 2 * REP  # 1920
            rem = NBIN - j0  # 81
            lo = (rem + 1) // 2  # 41
            hi = rem - lo  # 40
            nc.sync.dma_start(
                out=out_h[:, j0 : j0 + lo, :], in_=rep[0:BPC, 0:lo, :]
            )
            nc.scalar.dma_start(
                out=out_h[:, j0 + lo : j0 + rem, :], in_=rep[BPC:128, 0:hi, :]
            )

    nc.compile()
    return nc


def make_in_maps(chr_full: np.ndarray, ce: np.ndarray):
    chr_i16 = chr_full.astype(np.int16)
    ce_f32 = np.ascontiguousarray(ce.astype(np.float32))
    maps = []
    for c in range(N_CORES):
        shard = chr_i16[c * BPC : (c + 1) * BPC]
        # wrapped layout: idx i lives at [i % 16, i // 16], replicated to all
        # 8 GpSimd-core partition groups; indices 64..127 repeat the shard.
        wrapped = np.zeros((128, 8), dtype=np.int16)
        i = np.arange(128)
        wrapped[i % 16, i // 16] = shard[i % BPC]
        wrapped = np.tile(wrapped[:16, :], (8, 1))
        maps.append({"chr": np.ascontiguousarray(wrapped), "ce": ce_f32})
    return maps


def kernel(tensor=None, chr=None, ce=None, **_unused):
    chr_np = np.asarray(chr)
    ce_np = np.asarray(ce)
    nc = build_nc()
    res = run_bass_kernel_spmd(
        nc, make_in_maps(chr_np, ce_np), core_ids=list(range(N_CORES))
    )
    out = np.concatenate([r["out"] for r in res.results], axis=0)
    return out.astype(np.float32)


# revision 19
# speedup vs baseline: 5.4788x; 5.4788x over previous
"""ChromosomeEmbedding kernel for 8x Trainium2 NeuronCores.

Computes out[b, j, d] = ce[chr[b]-1, d] for b in [0,512), j in [0,2001),
d in [0,128). Data-parallel: the batch is sharded 64 samples/core across
8 cores; the tiny 24x128 table ce is replicated to every core.

Per-core device program (identical SPMD program on all cores):
  1. One DMA loads a packed prelude tensor: chr broadcast to [32, 128],
     an iota column (1..32), and the table zero-padded to 32 rows.
  2. One-hot gather on the tensor engine: onehotT[k, p] = (chr[p%64] == k+1)
     via a single is_equal tensor_scalar, then rows = onehotT.T @ ce as a
     fp32 matmul (exact -- exactly one 1.0 per one-hot column). Partition p
     of the PSUM result holds the embedding row of sample p % 64.
  3. Six doubling copies on the vector engine replicate each partition's
     row 64x along the free dim -> rep[128, 64, 128] (32 KB/partition).
  4. 33 large DMAs stream the [64, 2001, 128] output shard (65.5 MB).
     Partitions 0:64 are served by the even SDMA engines and 64:128 by the
     odd ones, so the low half is pinned to the sync HWDGE ring and the
     high half to the scalar ring (mixing rings lets them drift onto the
     same 8 engines and halves bandwidth). The sync ring walks bins
     [0, 1001) and the scalar ring [1001, 2001), keeping the two write
     streams far apart inside each sample's output region.

Measured on trn2: ~222 us HW exec for the 524 MB full output
(~325 GB/s/core steady-state HBM write rate), bit-exact vs reference.
"""

import functools

import numpy as np

from concourse import bacc, mybir, tile
from concourse.bass_utils import run_bass_kernel_spmd

N_CORES = 8
BS = 512
BPC = BS // N_CORES  # 64 samples per core
NBIN = 2001
DIM = 128
N_CHR = 24
KPAD = 32  # contraction dim: 24 table rows zero-padded to 32
REP = 64  # replicated copies of each row held in SBUF
PRE_W = 132 + DIM  # prelude row: 128 chr | iota | 3 pad | 128 table
F32 = mybir.dt.float32


@functools.lru_cache(maxsize=1)
def build_nc():
    nc = bacc.Bacc("TRN2", target_bir_lowering=False)

    pre_h = nc.declare_dram_parameter("pre", [KPAD, PRE_W], F32, isOutput=False)
    out_h = nc.declare_dram_parameter("out", [BPC, NBIN, DIM], F32, isOutput=True)

    with tile.TileContext(nc) as tc:
        with (
            tc.tile_pool(name="pool", bufs=1) as pool,
            tc.tile_pool(name="psum", bufs=1, space="PSUM") as psum,
        ):
            pre = pool.tile([KPAD, PRE_W], F32, tag="pre")
            oh = pool.tile([KPAD, 128], F32, tag="oh")
            rows_ps = psum.tile([128, DIM], F32, tag="rows")
            rep = pool.tile([128, REP, DIM], F32, tag="rep")

            # One-hot gather on the PE: onehotT[k, p] = (chr[p % 64] == k+1),
            # rows = onehotT.T @ ce (exact: one 1.0 per column). chr arrives
            # broadcast to [32, 128] (host-side replication, same as
            # replicating ce across cores).
            nc.sync.dma_start(out=pre[:, :], in_=pre_h[:, :])
            nc.vector.tensor_scalar(
                out=oh[:, :],
                in0=pre[:, 0:128],
                scalar1=pre[:, 128:129],
                scalar2=None,
                op0=mybir.AluOpType.is_equal,
            )
            nc.tensor.matmul(
                rows_ps[:, :],
                oh[:, :],
                pre[:, 132 : 132 + DIM],
                start=True,
                stop=True,
            )
            nc.vector.tensor_copy(out=rep[:, 0:1, :], in_=rows_ps[:, :])

            # Replicate each partition's row REP times along the free dim.
            w = 1
            while w < REP:
                nc.vector.tensor_copy(out=rep[:, w : 2 * w, :], in_=rep[:, 0:w, :])
                w *= 2

            # Stream the output. Partitions 0:64 map to the even DMA engines
            # and 64:128 to the odd ones, so pin each half to its own HWDGE
            # ring: sync walks bins [0, 1001), scalar walks [1001, 2001).
            half = (NBIN + 1) // 2  # 1001
            for t in range(half // REP):
                nc.sync.dma_start(
                    out=out_h[:, t * REP : (t + 1) * REP, :], in_=rep[0:BPC, :, :]
                )
            r0 = half % REP  # 41
            if r0:
                nc.sync.dma_start(
                    out=out_h[:, half - r0 : half, :], in_=rep[0:BPC, 0:r0, :]
                )
            for t in range((NBIN - half) // REP):
                nc.scalar.dma_start(
                    out=out_h[:, half + t * REP : half + (t + 1) * REP, :],
                    in_=rep[BPC:128, :, :],
                )
            r1 = (NBIN - half) % REP  # 40
            if r1:
                nc.scalar.dma_start(
                    out=out_h[:, NBIN - r1 : NBIN, :], in_=rep[BPC:128, 0:r1, :]
                )

    nc.compile()
    return nc


def make_in_maps(chr_full: np.ndarray, ce: np.ndarray):
    chr_f32 = chr_full.astype(np.float32)
    ce_pad = np.zeros((KPAD, DIM), np.float32)
    ce_pad[:N_CHR] = ce.astype(np.float32)
    maps = []
    for c in range(N_CORES):
        shard = chr_f32[c * BPC : (c + 1) * BPC]
        pre = np.zeros((KPAD, PRE_W), np.float32)
        pre[:, 0:128] = np.tile(shard, (KPAD, 2))  # chr broadcast
        pre[:, 128] = np.arange(1, KPAD + 1)  # iota
        pre[:, 132 : 132 + DIM] = ce_pad
        maps.append({"pre": np.ascontiguousarray(pre)})
    return maps


def kernel(tensor=None, chr=None, ce=None, **_unused):
    chr_np = np.asarray(chr)
    ce_np = np.asarray(ce)
    nc = build_nc()
    res = run_bass_kernel_spmd(
        nc, make_in_maps(chr_np, ce_np), core_ids=list(range(N_CORES))
    )
    out = np.concatenate([r["out"] for r in res.results], axis=0)
    return out.astype(np.float32)


# revision 26
# speedup vs baseline: 5.6866x; 1.0379x over previous
"""ChromosomeEmbedding kernel for 8x Trainium2 NeuronCores.

Computes out[b, j, d] = ce[chr[b]-1, d] for b in [0,512), j in [0,2001),
d in [0,128). Data-parallel: the batch is sharded 64 samples/core across
8 cores; the tiny 24x128 table ce is replicated to every core.

Per-core device program (identical SPMD program on all cores):
  1. One DMA loads a packed prelude tensor: chr broadcast to [32, 128],
     an iota column (1..32), and the table zero-padded to 32 rows.
  2. One-hot gather on the tensor engine: onehotT[k, p] = (chr[p%64] == k+1)
     via a single is_equal tensor_scalar, then rows = onehotT.T @ ce as a
     fp32 matmul (exact -- exactly one 1.0 per one-hot column). Partition p
     of the PSUM result holds the embedding row of sample p % 64.
  3. Six doubling copies on the vector engine replicate each partition's
     row 64x along the free dim -> rep[128, 64, 128] (32 KB/partition).
  4. ~34 large DMAs stream the [64, 2001, 128] output shard (65.5 MB),
     split between the two HWDGE rings: sync walks bins [0, 1001) from
     partitions 0:64, scalar walks [1001, 2001) from partitions 64:128.
     The SDMA engines round-robin between the two queue rings at packet
     granularity, so each ring opens with a 32-bin pass that depends only
     on the w=32 doubling copy -- both queues enter the engine rotation
     ~2 us earlier and the stream sustains ~340-360 GB/s instead of
     ~300 GB/s single-queue phases at the edges.

Measured on trn2: ~213 us HW exec for the 524 MB full output
(~345 GB/s/core steady-state HBM write rate), bit-exact vs reference.
"""

import functools

import numpy as np

from concourse import bacc, mybir, tile
from concourse.bass_utils import run_bass_kernel_spmd

N_CORES = 8
BS = 512
BPC = BS // N_CORES  # 64 samples per core
NBIN = 2001
DIM = 128
N_CHR = 24
KPAD = 32  # contraction dim: 24 table rows zero-padded to 32
REP = 64  # replicated copies of each row held in SBUF
PRE_W = 132 + DIM  # prelude row: 128 chr | iota | 3 pad | 128 table
SPLIT = 1001  # bins walked by the sync ring; scalar ring takes the rest
OPENERS = True  # small w=32-dependent opener pass per ring
F32 = mybir.dt.float32


@functools.lru_cache(maxsize=1)
def build_nc():
    nc = bacc.Bacc("TRN2", target_bir_lowering=False)

    pre_h = nc.declare_dram_parameter("pre", [KPAD, PRE_W], F32, isOutput=False)
    out_h = nc.declare_dram_parameter("out", [BPC, NBIN, DIM], F32, isOutput=True)

    with tile.TileContext(nc) as tc:
        with (
            tc.tile_pool(name="pool", bufs=1) as pool,
            tc.tile_pool(name="psum", bufs=1, space="PSUM") as psum,
        ):
            pre = pool.tile([KPAD, PRE_W], F32, tag="pre")
            oh = pool.tile([KPAD, 128], F32, tag="oh")
            rows_ps = psum.tile([128, DIM], F32, tag="rows")
            rep = pool.tile([128, REP, DIM], F32, tag="rep")

            # One-hot gather on the PE: onehotT[k, p] = (chr[p % 64] == k+1),
            # rows = onehotT.T @ ce (exact: one 1.0 per column). chr arrives
            # broadcast to [32, 128] (host-side replication, same as
            # replicating ce across cores).
            nc.sync.dma_start(out=pre[:, :], in_=pre_h[:, :])
            nc.vector.tensor_scalar(
                out=oh[:, :],
                in0=pre[:, 0:128],
                scalar1=pre[:, 128:129],
                scalar2=None,
                op0=mybir.AluOpType.is_equal,
            )
            nc.tensor.matmul(
                rows_ps[:, :],
                oh[:, :],
                pre[:, 132 : 132 + DIM],
                start=True,
                stop=True,
            )
            nc.vector.tensor_copy(out=rep[:, 0:1, :], in_=rows_ps[:, :])

            # Replicate each partition's row REP times along the free dim.
            w = 1
            while w < REP:
                nc.vector.tensor_copy(out=rep[:, w : 2 * w, :], in_=rep[:, 0:w, :])
                w *= 2

            # Stream the output. The low partition half is pinned to the
            # sync HWDGE ring and the high half to the scalar ring; the sync
            # ring enters the engine rotation ~8 us earlier (it also carries
            # the prelude DMA), so it gets more bins so both rings drain dry
            # at the same time.
            half = SPLIT
            OPEN = 32 if OPENERS else 0
            if OPEN:
                # 32-bin openers depend only on the w=32 copy, putting both
                # queues into the SDMA engines' rotation ~2 us earlier.
                nc.sync.dma_start(
                    out=out_h[:, 0:OPEN, :], in_=rep[0:BPC, 0:OPEN, :]
                )
                nc.scalar.dma_start(
                    out=out_h[:, half : half + OPEN, :], in_=rep[BPC:128, 0:OPEN, :]
                )
            for t in range((half - OPEN) // REP):
                nc.sync.dma_start(
                    out=out_h[:, OPEN + t * REP : OPEN + (t + 1) * REP, :],
                    in_=rep[0:BPC, :, :],
                )
            r0 = (half - OPEN) % REP
            if r0:
                nc.sync.dma_start(
                    out=out_h[:, half - r0 : half, :], in_=rep[0:BPC, 0:r0, :]
                )
            for t in range((NBIN - half - OPEN) // REP):
                nc.scalar.dma_start(
                    out=out_h[
                        :, half + OPEN + t * REP : half + OPEN + (t + 1) * REP, :
                    ],
                    in_=rep[BPC:128, :, :],
                )
            r1 = (NBIN - half - OPEN) % REP
            if r1:
                nc.scalar.dma_start(
                    out=out_h[:, NBIN - r1 : NBIN, :], in_=rep[BPC:128, 0:r1, :]
                )

    nc.compile()
    return nc


def make_in_maps(chr_full: np.ndarray, ce: np.ndarray):
    chr_f32 = chr_full.astype(np.float32)
    ce_pad = np.zeros((KPAD, DIM), np.float32)
    ce_pad[:N_CHR] = ce.astype(np.float32)
    maps = []
    for c in range(N_CORES):
        shard = chr_f32[c * BPC : (c + 1) * BPC]
        pre = np.zeros((KPAD, PRE_W), np.float32)
        pre[:, 0:128] = np.tile(shard, (KPAD, 2))  # chr broadcast
        pre[:, 128] = np.arange(1, KPAD + 1)  # iota
        pre[:, 132 : 132 + DIM] = ce_pad
        maps.append({"pre": np.ascontiguousarray(pre)})
    return maps


def kernel(tensor=None, chr=None, ce=None, **_unused):
    chr_np = np.asarray(chr)
    ce_np = np.asarray(ce)
    nc = build_nc()
    res = run_bass_kernel_spmd(
        nc, make_in_maps(chr_np, ce_np), core_ids=list(range(N_CORES))
    )
    out = np.concatenate([r["out"] for r in res.results], axis=0)
    return out.astype(np.float32)
